# revision 29
# baseline (speedup 1.0000x reference)
"""HMU-layer (omega) Trainium2 kernel.

out[b,n] = exp(-(lam_n*||x_b-mu_n||^2 + sum_k om_nk*((x_b-mu_n)@v_nk)^2)/D)

Strategy (tensor-parallel over n, 8 cores, full I/O):
  Host folds all weight-only terms (fp32) AND the x-side prep:
    vt  = bf16(sqrt(om)*v) laid out k-major (d, k*NLOC+n)
    G   = -2*lam*muc - 2*sum_k r*vq   (d, n) bf16
    C   = lam*|muc|^2 + sum_k r^2     split hi/lo bf16
    xT  = bf16(x-0.5).T  (d, b);  xc2 = rowsum(xT^2) bf16
  Device per core (n_loc=1024), chunk-major staggered schedule:
    64 steps of (b-tile i, k-plane m):
      y[b, n] = xT_i @ v_m        (PE, 4 MMs of N=512)
      z = y^2                     (ACT Square / DVE copy+mul, split tunable)
      pair/quad adds (DVE, bf16 2x)  -> s[b,n] = sum_k z
    per-completion: w = xc@G + lam*xc2 + C (PE, at plane 6)
      q3 = s + w (DVE), out = exp(-q3/256) (ACT), DMA out.
  Startup: DMAs ordered x/G/lc then v0..v7; exp table preloaded; dummy
  matmuls warm the PE HAM clock-gate during the DMA window.
"""
import sys

sys.path.insert(0, "/opt/trn_rl_repo")

from contextlib import ExitStack

import ml_dtypes
import numpy as np

import concourse.bass as bass
import concourse.tile as tile
from concourse import bacc, mybir
from concourse.bass_utils import run_bass_kernel_spmd

B, N, D, K = 1024, 8192, 256, 8
NCORES = 8
NLOC = N // NCORES          # 1024 units per core
NKLOC = NLOC * K            # 8192
F32 = mybir.dt.float32
BF16 = mybir.dt.bfloat16
BF = ml_dtypes.bfloat16

# ---- tuning knobs ----
YBUFS = 4                   # shared y/w psum tiles (2 banks each, 8 total)
N_WARM_MM = 40              # dummy matmuls to warm the PE HAM clock gate
# square engine split: cols of each k-plane squared on DVE (rest on ACT).
DVE_SQ_COLS = {7: 1024}
# pair-level reduction adds run on SWDGE DMA engines (bf16 in-place accum)
DMA_PAIRS = False
# in the last TAIL_SPLIT steps, split squares ACT/DVE to drain both engines
TAIL_SPLIT = 0
TAIL_DVE_COLS = 384
W_STEP = 6
# completion-stagger targets for the 64-step schedule
COMPLETIONS = [27, 32, 37, 42, 47, 52, 57, 63]


SPREAD = 1.25


def build_schedule():
    """64 (i, m) steps: earliest-deadline-first with v-DMA availability.

    Per-chunk deadlines spread each b-tile's planes across the timeline so
    no b-tile's work piles up at the end (short serial tail).
    """
    avail = [0] + [int(1.6 * j) + 1 for j in range(1, K)]
    nxt = [0] * 8
    sched = []
    for s in range(64):
        best, best_d = None, None
        for i in range(8):
            m = nxt[i]
            if m >= K or s < avail[m]:
                continue
            d = COMPLETIONS[i] - SPREAD * (K - 1 - m)
            if best_d is None or d < best_d:
                best, best_d = i, d
        if best is None:  # availability too strict; take any unfinished
            for i in range(8):
                if nxt[i] < K:
                    best = i
                    break
        sched.append((best, nxt[best]))
        nxt[best] += 1
    assert all(v == K for v in nxt)
    return sched


SCHED = build_schedule()


def _kernel_body(tc, out, xT_d, xc2_d, vt_d, gt_d, lc_d, loop_t=1):
    nc = tc.nc
    act = mybir.ActivationFunctionType
    with ExitStack() as ctx:
        weights = ctx.enter_context(tc.tile_pool(name="weights", bufs=1))
        zpool = ctx.enter_context(tc.tile_pool(name="zpool", bufs=10))
        ppool = ctx.enter_context(tc.tile_pool(name="ppool", bufs=10))
        qpool = ctx.enter_context(tc.tile_pool(name="qpool", bufs=10))
        spool = ctx.enter_context(tc.tile_pool(name="spool", bufs=2))
        mpool = ctx.enter_context(tc.tile_pool(name="mpool", bufs=2))
        opool = ctx.enter_context(tc.tile_pool(name="opool", bufs=2))
        ybpool = ctx.enter_context(tc.tile_pool(name="ybpool", bufs=2))
        ypsum = ctx.enter_context(
            tc.tile_pool(name="ypsum", bufs=YBUFS, space="PSUM")
        )

        # ---- HAM warm-up scratch first (unblocks dummy matmuls ASAP) ----
        if N_WARM_MM:
            wsc = weights.tile([128, 128], BF16, tag="wsc")
            nc.vector.memset(wsc, 0.0)
            warm = ypsum.tile([128, NLOC], F32, tag="y")
            for _ in range(N_WARM_MM):
                nc.tensor.matmul(
                    warm[:, 0:128], lhsT=wsc, rhs=wsc, start=True, stop=True
                )
        # exp table preload (overlaps the DMA window)
        esc = weights.tile([1, 8], F32, tag="esc")
        nc.vector.memset(esc, 0.0)
        esco = weights.tile([1, 8], F32, tag="esco")
        nc.scalar.activation(out=esco, in_=esc, func=act.Exp)

        # ---- resident inputs; DMA issue order = need order.
        # Inputs split across two HWDGE rings (SP + ACT) so xT/v0 land
        # in parallel; in-loop output DMAs stay on SP.
        xT = weights.tile([128, 2, B], BF16, tag="xT")
        for h in range(2):
            nc.sync.dma_start(out=xT[:, h, :], in_=xT_d[h * 128 : (h + 1) * 128, :])
        v_tiles = []
        for j in range(K):
            vtile = weights.tile([128, 2, NLOC], BF16, tag=f"v{j}")
            v_tiles.append(vtile)

        def load_v(j, eng):
            for h in range(2):
                eng.dma_start(
                    out=v_tiles[j][:, h, :],
                    in_=vt_d[h * 128 : (h + 1) * 128, j * NLOC : (j + 1) * NLOC],
                )

        load_v(0, nc.sync)
        g_sb = weights.tile([128, 2, NLOC], BF16, tag="g")
        for h in range(2):
            nc.sync.dma_start(out=g_sb[:, h, :], in_=gt_d[h * 128 : (h + 1) * 128, :])
        lc_sb = weights.tile([3, NLOC], BF16, tag="lc")
        nc.sync.dma_start(out=lc_sb, in_=lc_d)
        xc2ones = weights.tile([3, B], BF16, tag="xc2")
        nc.vector.memset(xc2ones, 1.0)
        nc.sync.dma_start(out=xc2ones[0:1, :], in_=xc2_d)
        for j in range(1, K):
            load_v(j, nc.sync)

        # ---- main loop: 64 staggered (b-tile, k-plane) steps ----
        if loop_t > 1:
            loop_cm = tc.For_i(0, loop_t, 1, hint_engines=(mybir.EngineType.PE,))
            loop_cm.__enter__()

        z_prev = {}   # i -> z tile of the even plane awaiting its pair
        pairs = {}    # (i, pair_idx) -> pair-sum tile
        quad01 = {}   # i -> quad01 tile
        wp_ref = {}   # i -> w psum tile
        for step, (i, m) in enumerate(SCHED):
            last = step == len(SCHED) - 1
            bs = slice(i * 128, (i + 1) * 128)
            # y = xc_i @ v_m
            yp = ypsum.tile([128, NLOC], F32, tag="y")
            for half in range(2):
                ysl = slice(half * 512, (half + 1) * 512)
                for h in range(2):
                    nc.tensor.matmul(
                        yp[:, ysl],
                        lhsT=xT[:, h, bs],
                        rhs=v_tiles[m][:, h, ysl],
                        start=(h == 0),
                        stop=(h == 1),
                    )
            # square into z (ACT head / DVE tail split)
            z = zpool.tile([128, NLOC], BF16, tag="z")
            if not last:
                if step >= len(SCHED) - 1 - TAIL_SPLIT:
                    dcols = TAIL_DVE_COLS
                else:
                    dcols = DVE_SQ_COLS.get(m, 0)
                acols = NLOC - dcols
                if acols:
                    nc.scalar.activation(
                        out=z[:, 0:acols], in_=yp[:, 0:acols], func=act.Square
                    )
                if dcols:
                    yb = ybpool.tile([128, dcols], BF16, tag="yb")
                    nc.vector.tensor_copy(out=yb, in_=yp[:, acols:NLOC])
                    nc.vector.tensor_mul(out=z[:, acols:NLOC], in0=yb, in1=yb)
            # reduction: pairs (0,1)(2,3)(4,5)(6,7) -> quads -> s
            if m % 2 == 0:
                z_prev[i] = z
            elif not last:
                if DMA_PAIRS:
                    # in-place accumulate odd plane into the even plane's
                    # tile on the (otherwise idle) DMA engines
                    p = z_prev.pop(i)
                    nc.gpsimd.dma_start(
                        out=p, in_=z, accum_op=mybir.AluOpType.add
                    )
                else:
                    p = ppool.tile([128, NLOC], BF16, tag="p")
                    nc.vector.tensor_add(out=p, in0=z_prev.pop(i), in1=z)
                pairs[(i, m // 2)] = p
            if m == 3:
                q01 = qpool.tile([128, NLOC], BF16, tag="q01")
                nc.vector.tensor_add(
                    out=q01, in0=pairs.pop((i, 0)), in1=pairs.pop((i, 1))
                )
                quad01[i] = q01
            if m == W_STEP:
                # w = xc@G + lam*xc2 + C (consumed at completion, next step)
                wp = ypsum.tile([128, NLOC], F32, tag="y")
                for nkh in range(2):
                    sl = slice(nkh * 512, (nkh + 1) * 512)
                    for h in range(2):
                        nc.tensor.matmul(
                            wp[:, sl],
                            lhsT=xT[:, h, bs],
                            rhs=g_sb[:, h, sl],
                            start=(h == 0),
                            stop=False,
                        )
                    nc.tensor.matmul(
                        wp[:, sl],
                        lhsT=xc2ones[:, bs],
                        rhs=lc_sb[:, sl],
                        start=False,
                        stop=True,
                    )
                wp_ref[i] = wp
            if m == 7:
                p2 = pairs.pop((i, 2))
                if last:
                    p3 = ppool.tile([128, NLOC], BF16, tag="p")
                else:
                    p3 = pairs.pop((i, 3))
                q01, wp_i = quad01.pop(i), wp_ref.pop(i)
                q23 = qpool.tile([128, NLOC], BF16, tag="q23")
                st = spool.tile([128, NLOC], BF16, tag="s")
                q3 = mpool.tile([128, NLOC], F32, tag="q3")
                o = opool.tile([128, NLOC], F32, tag="o")
                # last completion: split the serial finale into halves so the
                # tail chain (pair->quad->s->q3->exp->DMA) pipelines
                halves = (
                    (slice(0, 512), slice(512, NLOC)) if last else (slice(0, NLOC),)
                )
                for hs in halves:
                    if last:
                        # split square on ACT so each half chains immediately
                        nc.scalar.activation(
                            out=z[:, hs], in_=yp[:, hs], func=act.Square
                        )
                        nc.vector.tensor_add(
                            out=p3[:, hs], in0=z_prev[i][:, hs], in1=z[:, hs]
                        )
                        if hs.stop == NLOC:
                            z_prev.pop(i)
                    nc.vector.tensor_add(out=q23[:, hs], in0=p2[:, hs], in1=p3[:, hs])
                    nc.vector.tensor_add(out=st[:, hs], in0=q01[:, hs], in1=q23[:, hs])
                    nc.vector.tensor_add(out=q3[:, hs], in0=st[:, hs], in1=wp_i[:, hs])
                    nc.scalar.activation(
                        out=o[:, hs], in_=q3[:, hs], func=act.Exp, scale=-1.0 / D
                    )
                    oeng = nc.sync if last else nc.gpsimd
                    oeng.dma_start(out=out[bs, hs], in_=o[:, hs])

        if loop_t > 1:
            loop_cm.__exit__(None, None, None)


_NC_CACHE = {}


def _build(loop_t=1):
    if loop_t in _NC_CACHE:
        return _NC_CACHE[loop_t]
    nc = bacc.Bacc("TRN2", target_bir_lowering=False, debug=False)
    xT_da = nc.dram_tensor("xT_in", (D, B), BF16, kind="ExternalInput").ap()
    xc2_d = nc.dram_tensor("xc2_in", (1, B), BF16, kind="ExternalInput").ap()
    vt_d = nc.dram_tensor("vt_in", (D, NKLOC), BF16, kind="ExternalInput").ap()
    gt_d = nc.dram_tensor("gt_in", (D, NLOC), BF16, kind="ExternalInput").ap()
    lc_d = nc.dram_tensor("lc_in", (3, NLOC), BF16, kind="ExternalInput").ap()
    out_d = nc.dram_tensor("out", (B, NLOC), F32, kind="ExternalOutput").ap()
    with tile.TileContext(nc) as tc:
        _kernel_body(tc, out_d, xT_da, xc2_d, vt_d, gt_d, lc_d, loop_t=loop_t)
    nc.compile()
    _NC_CACHE[loop_t] = nc
    return nc


def _host_fold(x, mu, lambda_base, v, omega):
    """Weight-only + x-side folding and sharding. Returns per-core inputs."""
    x = np.ascontiguousarray(x, dtype=np.float32)
    xcb = (x - 0.5).astype(BF)                       # (B, D) bf16
    xT = np.ascontiguousarray(xcb.T)                 # (D, B)
    xc2 = (xcb.astype(np.float32) ** 2).sum(-1)      # (B,) f32
    xc2_row = np.ascontiguousarray(xc2.astype(BF)[None, :])
    in_maps = []
    for c in range(NCORES):
        sl = slice(c * NLOC, (c + 1) * NLOC)
        mu_c = mu[sl].astype(np.float32)
        lam_c = lambda_base[sl].astype(np.float32)
        v_c = v[sl].astype(np.float32)
        om_c = omega[sl].astype(np.float32)
        vt = np.sqrt(om_c)[:, :, None] * v_c            # (NLOC, K, D)
        vt_bf = vt.astype(BF)
        vq = vt_bf.astype(np.float32)
        t = 0.5 * vq.sum(-1)                            # (NLOC, K)
        m = np.einsum("nd,nkd->nk", mu_c, vq)
        r = m - t
        muc = mu_c - 0.5
        G = -2.0 * lam_c[:, None] * muc - 2.0 * np.einsum("nk,nkd->nd", r, vq)
        C = lam_c * (muc**2).sum(-1) + (r**2).sum(-1)
        C_hi = C.astype(BF)
        C_lo = (C - C_hi.astype(np.float32)).astype(BF)
        lc_rows = np.stack(
            [lam_c.astype(BF), C_hi, C_lo], axis=0
        )                                               # (3, NLOC) bf16
        # vt layout k-major (D, K*NLOC): vt_t[d, k*NLOC+n] = vt_bf[n, k, d]
        vt_t = np.ascontiguousarray(vt_bf.transpose(2, 1, 0).reshape(D, NKLOC))
        gt = np.ascontiguousarray(G.T.astype(BF))       # (D, NLOC)
        in_maps.append(
            {
                "xT_in": xT,
                "xc2_in": xc2_row,
                "vt_in": vt_t,
                "gt_in": gt,
                "lc_in": lc_rows,
            }
        )
    return in_maps


def kernel(x, mu, lambda_base, v, omega, _trace=False, _trace_kwargs=None):
    nc = _build()
    in_maps = _host_fold(x, mu, lambda_base, v, omega)
    res = run_bass_kernel_spmd(
        nc,
        in_maps,
        core_ids=list(range(NCORES)),
        trace=_trace,
        **(_trace_kwargs or {}),
    )
    out = np.concatenate(
        [res.results[c]["out"] for c in range(NCORES)], axis=1
    ).astype(np.float32)
    if _trace:
        kernel._last_result = res
    return out


# revision 37
# speedup vs baseline: 1.0066x; 1.0066x over previous
"""HMU-layer (omega) Trainium2 kernel.

out[b,n] = exp(-(lam_n*||x_b-mu_n||^2 + sum_k om_nk*((x_b-mu_n)@v_nk)^2)/D)

Strategy (tensor-parallel over n, 8 cores, full I/O):
  Host folds all weight-only terms (fp32) AND the x-side prep:
    vt  = bf16(sqrt(om)*v) laid out k-major (d, k*NLOC+n)
    G   = -2*lam*muc - 2*sum_k r*vq   (d, n) bf16
    C   = lam*|muc|^2 + sum_k r^2     split hi/lo bf16
    xT  = bf16(x-0.5).T  (d, b);  xc2 = rowsum(xT^2) bf16
  Device per core (n_loc=1024), chunk-major staggered schedule:
    64 steps of (b-tile i, k-plane m):
      y[b, n] = xT_i @ v_m        (PE, 4 MMs of N=512)
      z = y^2                     (ACT Square / DVE copy+mul, split tunable)
      pair/quad adds (DVE, bf16 2x)  -> s[b,n] = sum_k z
    per-completion: w = xc@G + lam*xc2 + C (PE, at plane 6)
      q3 = s + w (DVE), out = exp(-q3/256) (ACT), DMA out.
  Startup: DMAs ordered x/G/lc then v0..v7; exp table preloaded; dummy
  matmuls warm the PE HAM clock-gate during the DMA window.
"""
import sys

sys.path.insert(0, "/opt/trn_rl_repo")

from contextlib import ExitStack

import ml_dtypes
import numpy as np

import concourse.bass as bass
import concourse.tile as tile
from concourse import bacc, mybir
from concourse.bass_utils import run_bass_kernel_spmd

B, N, D, K = 1024, 8192, 256, 8
NCORES = 8
NLOC = N // NCORES          # 1024 units per core
NKLOC = NLOC * K            # 8192
F32 = mybir.dt.float32
BF16 = mybir.dt.bfloat16
BF = ml_dtypes.bfloat16

# ---- tuning knobs ----
YBUFS = 4                   # shared y/w psum tiles (2 banks each, 8 total)
N_WARM_MM = 40              # dummy matmuls to warm the PE HAM clock gate
# square engine split: cols of each k-plane squared on DVE (rest on ACT).
DVE_SQ_COLS = {7: 1024}
# pair-level reduction adds run on SWDGE DMA engines (bf16 in-place accum)
DMA_PAIRS = False
# in the last TAIL_SPLIT steps, split squares ACT/DVE to drain both engines
TAIL_SPLIT = 0
TAIL_DVE_COLS = 384
W_STEP = 6
# completion-stagger targets for the 64-step schedule
COMPLETIONS = [27, 32, 37, 42, 47, 52, 57, 63]


SPREAD = 1.25
AVAIL_COEF = 1.6


def build_schedule():
    """64 (i, m) steps: earliest-deadline-first with v-DMA availability.

    Per-chunk deadlines spread each b-tile's planes across the timeline so
    no b-tile's work piles up at the end (short serial tail).
    """
    avail = [0] + [int(AVAIL_COEF * j) + 1 for j in range(1, K)]
    nxt = [0] * 8
    sched = []
    for s in range(64):
        best, best_d = None, None
        for i in range(8):
            m = nxt[i]
            if m >= K or s < avail[m]:
                continue
            d = COMPLETIONS[i] - SPREAD * (K - 1 - m)
            if best_d is None or d < best_d:
                best, best_d = i, d
        if best is None:  # availability too strict; take any unfinished
            for i in range(8):
                if nxt[i] < K:
                    best = i
                    break
        sched.append((best, nxt[best]))
        nxt[best] += 1
    assert all(v == K for v in nxt)
    return sched


SCHED = build_schedule()


def _kernel_body(tc, out, xT_d, xc2_d, vt_d, gt_d, lc_d, loop_t=1):
    nc = tc.nc
    act = mybir.ActivationFunctionType
    with ExitStack() as ctx:
        weights = ctx.enter_context(tc.tile_pool(name="weights", bufs=1))
        zpool = ctx.enter_context(tc.tile_pool(name="zpool", bufs=10))
        ppool = ctx.enter_context(tc.tile_pool(name="ppool", bufs=10))
        qpool = ctx.enter_context(tc.tile_pool(name="qpool", bufs=10))
        spool = ctx.enter_context(tc.tile_pool(name="spool", bufs=2))
        mpool = ctx.enter_context(tc.tile_pool(name="mpool", bufs=2))
        opool = ctx.enter_context(tc.tile_pool(name="opool", bufs=2))
        ybpool = ctx.enter_context(tc.tile_pool(name="ybpool", bufs=2))
        ypsum = ctx.enter_context(
            tc.tile_pool(name="ypsum", bufs=YBUFS, space="PSUM")
        )

        # ---- HAM warm-up scratch first (unblocks dummy matmuls ASAP) ----
        if N_WARM_MM:
            wsc = weights.tile([128, 128], BF16, tag="wsc")
            nc.vector.memset(wsc, 0.0)
            warm = ypsum.tile([128, NLOC], F32, tag="y")
            for _ in range(N_WARM_MM):
                nc.tensor.matmul(
                    warm[:, 0:128], lhsT=wsc, rhs=wsc, start=True, stop=True
                )
        # exp table preload (overlaps the DMA window)
        esc = weights.tile([1, 8], F32, tag="esc")
        nc.vector.memset(esc, 0.0)
        esco = weights.tile([1, 8], F32, tag="esco")
        nc.scalar.activation(out=esco, in_=esc, func=act.Exp)

        # ---- resident inputs; DMA issue order = need order.
        # Inputs split across two HWDGE rings (SP + ACT) so xT/v0 land
        # in parallel; in-loop output DMAs stay on SP.
        xT = weights.tile([128, 2, B], BF16, tag="xT")
        for h in range(2):
            nc.sync.dma_start(out=xT[:, h, :], in_=xT_d[h * 128 : (h + 1) * 128, :])
        v_tiles = []
        for j in range(K):
            vtile = weights.tile([128, 2, NLOC], BF16, tag=f"v{j}")
            v_tiles.append(vtile)

        def load_v(j, eng, split=False):
            css = (slice(0, 512), slice(512, NLOC)) if split else (slice(0, NLOC),)
            for cs in css:
                for h in range(2):
                    eng.dma_start(
                        out=v_tiles[j][:, h, cs],
                        in_=vt_d[
                            h * 128 : (h + 1) * 128,
                            j * NLOC + (cs.start or 0) : j * NLOC + cs.stop,
                        ],
                    )

        load_v(0, nc.sync)
        load_v(1, nc.sync)
        load_v(2, nc.sync)
        load_v(3, nc.sync)
        load_v(4, nc.sync)
        g_sb = weights.tile([128, 2, NLOC], BF16, tag="g")
        for h in range(2):
            nc.sync.dma_start(out=g_sb[:, h, :], in_=gt_d[h * 128 : (h + 1) * 128, :])
        lc_sb = weights.tile([3, NLOC], BF16, tag="lc")
        nc.sync.dma_start(out=lc_sb, in_=lc_d)
        xc2ones = weights.tile([3, B], BF16, tag="xc2")
        nc.vector.memset(xc2ones, 1.0)
        nc.sync.dma_start(out=xc2ones[0:1, :], in_=xc2_d)
        for j in range(5, K):
            load_v(j, nc.sync)

        # ---- main loop: 64 staggered (b-tile, k-plane) steps ----
        if loop_t > 1:
            loop_cm = tc.For_i(0, loop_t, 1, hint_engines=(mybir.EngineType.PE,))
            loop_cm.__enter__()

        z_prev = {}   # i -> z tile of the even plane awaiting its pair
        pairs = {}    # (i, pair_idx) -> pair-sum tile
        quad01 = {}   # i -> quad01 tile
        wp_ref = {}   # i -> w psum tile
        for step, (i, m) in enumerate(SCHED):
            last = step == len(SCHED) - 1
            bs = slice(i * 128, (i + 1) * 128)
            # y = xc_i @ v_m
            yp = ypsum.tile([128, NLOC], F32, tag="y")
            for half in range(2):
                ysl = slice(half * 512, (half + 1) * 512)
                for h in range(2):
                    nc.tensor.matmul(
                        yp[:, ysl],
                        lhsT=xT[:, h, bs],
                        rhs=v_tiles[m][:, h, ysl],
                        start=(h == 0),
                        stop=(h == 1),
                    )
            # square into z (ACT head / DVE tail split)
            z = zpool.tile([128, NLOC], BF16, tag="z")
            if not last:
                if step >= len(SCHED) - 1 - TAIL_SPLIT:
                    dcols = TAIL_DVE_COLS
                else:
                    dcols = DVE_SQ_COLS.get(m, 0)
                acols = NLOC - dcols
                if acols:
                    nc.scalar.activation(
                        out=z[:, 0:acols], in_=yp[:, 0:acols], func=act.Square
                    )
                if dcols:
                    yb = ybpool.tile([128, dcols], BF16, tag="yb")
                    nc.vector.tensor_copy(out=yb, in_=yp[:, acols:NLOC])
                    nc.vector.tensor_mul(out=z[:, acols:NLOC], in0=yb, in1=yb)
            # reduction: pairs (0,1)(2,3)(4,5)(6,7) -> quads -> s
            if m % 2 == 0:
                z_prev[i] = z
            elif not last:
                if DMA_PAIRS:
                    # in-place accumulate odd plane into the even plane's
                    # tile on the (otherwise idle) DMA engines
                    p = z_prev.pop(i)
                    nc.gpsimd.dma_start(
                        out=p, in_=z, accum_op=mybir.AluOpType.add
                    )
                else:
                    p = ppool.tile([128, NLOC], BF16, tag="p")
                    nc.vector.tensor_add(out=p, in0=z_prev.pop(i), in1=z)
                pairs[(i, m // 2)] = p
            if m == 3:
                q01 = qpool.tile([128, NLOC], BF16, tag="q01")
                nc.vector.tensor_add(
                    out=q01, in0=pairs.pop((i, 0)), in1=pairs.pop((i, 1))
                )
                quad01[i] = q01
            if m == W_STEP:
                # w = xc@G + lam*xc2 + C (consumed at completion, next step)
                wp = ypsum.tile([128, NLOC], F32, tag="y")
                for nkh in range(2):
                    sl = slice(nkh * 512, (nkh + 1) * 512)
                    for h in range(2):
                        nc.tensor.matmul(
                            wp[:, sl],
                            lhsT=xT[:, h, bs],
                            rhs=g_sb[:, h, sl],
                            start=(h == 0),
                            stop=False,
                        )
                    nc.tensor.matmul(
                        wp[:, sl],
                        lhsT=xc2ones[:, bs],
                        rhs=lc_sb[:, sl],
                        start=False,
                        stop=True,
                    )
                wp_ref[i] = wp
            if m == 7:
                p2 = pairs.pop((i, 2))
                if last:
                    p3 = ppool.tile([128, NLOC], BF16, tag="p")
                else:
                    p3 = pairs.pop((i, 3))
                q01, wp_i = quad01.pop(i), wp_ref.pop(i)
                q23 = qpool.tile([128, NLOC], BF16, tag="q23")
                st = spool.tile([128, NLOC], BF16, tag="s")
                q3 = mpool.tile([128, NLOC], F32, tag="q3")
                o = opool.tile([128, NLOC], F32, tag="o")
                # last completion: split the serial finale into halves so the
                # tail chain (pair->quad->s->q3->exp->DMA) pipelines
                halves = (
                    (slice(0, 512), slice(512, NLOC)) if last else (slice(0, NLOC),)
                )
                for hs in halves:
                    if last:
                        # split square on ACT so each half chains immediately
                        nc.scalar.activation(
                            out=z[:, hs], in_=yp[:, hs], func=act.Square
                        )
                        nc.vector.tensor_add(
                            out=p3[:, hs], in0=z_prev[i][:, hs], in1=z[:, hs]
                        )
                        if hs.stop == NLOC:
                            z_prev.pop(i)
                    nc.vector.tensor_add(out=q23[:, hs], in0=p2[:, hs], in1=p3[:, hs])
                    nc.vector.tensor_add(out=st[:, hs], in0=q01[:, hs], in1=q23[:, hs])
                    nc.vector.tensor_add(out=q3[:, hs], in0=st[:, hs], in1=wp_i[:, hs])
                    nc.scalar.activation(
                        out=o[:, hs], in_=q3[:, hs], func=act.Exp, scale=-1.0 / D
                    )
                    oeng = nc.sync if last else nc.gpsimd
                    oeng.dma_start(out=out[bs, hs], in_=o[:, hs])

        if loop_t > 1:
            loop_cm.__exit__(None, None, None)


_NC_CACHE = {}


def _build(loop_t=1):
    if loop_t in _NC_CACHE:
        return _NC_CACHE[loop_t]
    nc = bacc.Bacc("TRN2", target_bir_lowering=False, debug=False)
    xT_da = nc.dram_tensor("xT_in", (D, B), BF16, kind="ExternalInput").ap()
    xc2_d = nc.dram_tensor("xc2_in", (1, B), BF16, kind="ExternalInput").ap()
    vt_d = nc.dram_tensor("vt_in", (D, NKLOC), BF16, kind="ExternalInput").ap()
    gt_d = nc.dram_tensor("gt_in", (D, NLOC), BF16, kind="ExternalInput").ap()
    lc_d = nc.dram_tensor("lc_in", (3, NLOC), BF16, kind="ExternalInput").ap()
    out_d = nc.dram_tensor("out", (B, NLOC), F32, kind="ExternalOutput").ap()
    with tile.TileContext(nc) as tc:
        _kernel_body(tc, out_d, xT_da, xc2_d, vt_d, gt_d, lc_d, loop_t=loop_t)
    nc.compile()
    _NC_CACHE[loop_t] = nc
    return nc


def _host_fold(x, mu, lambda_base, v, omega):
    """Weight-only + x-side folding and sharding. Returns per-core inputs."""
    x = np.ascontiguousarray(x, dtype=np.float32)
    xcb = (x - 0.5).astype(BF)                       # (B, D) bf16
    xT = np.ascontiguousarray(xcb.T)                 # (D, B)
    xc2 = (xcb.astype(np.float32) ** 2).sum(-1)      # (B,) f32
    xc2_row = np.ascontiguousarray(xc2.astype(BF)[None, :])
    in_maps = []
    for c in range(NCORES):
        sl = slice(c * NLOC, (c + 1) * NLOC)
        mu_c = mu[sl].astype(np.float32)
        lam_c = lambda_base[sl].astype(np.float32)
        v_c = v[sl].astype(np.float32)
        om_c = omega[sl].astype(np.float32)
        vt = np.sqrt(om_c)[:, :, None] * v_c            # (NLOC, K, D)
        vt_bf = vt.astype(BF)
        vq = vt_bf.astype(np.float32)
        t = 0.5 * vq.sum(-1)                            # (NLOC, K)
        m = np.einsum("nd,nkd->nk", mu_c, vq)
        r = m - t
        muc = mu_c - 0.5
        G = -2.0 * lam_c[:, None] * muc - 2.0 * np.einsum("nk,nkd->nd", r, vq)
        C = lam_c * (muc**2).sum(-1) + (r**2).sum(-1)
        C_hi = C.astype(BF)
        C_lo = (C - C_hi.astype(np.float32)).astype(BF)
        lc_rows = np.stack(
            [lam_c.astype(BF), C_hi, C_lo], axis=0
        )                                               # (3, NLOC) bf16
        # vt layout k-major (D, K*NLOC): vt_t[d, k*NLOC+n] = vt_bf[n, k, d]
        vt_t = np.ascontiguousarray(vt_bf.transpose(2, 1, 0).reshape(D, NKLOC))
        gt = np.ascontiguousarray(G.T.astype(BF))       # (D, NLOC)
        in_maps.append(
            {
                "xT_in": xT,
                "xc2_in": xc2_row,
                "vt_in": vt_t,
                "gt_in": gt,
                "lc_in": lc_rows,
            }
        )
    return in_maps


def kernel(x, mu, lambda_base, v, omega, _trace=False, _trace_kwargs=None):
    nc = _build()
    in_maps = _host_fold(x, mu, lambda_base, v, omega)
    res = run_bass_kernel_spmd(
        nc,
        in_maps,
        core_ids=list(range(NCORES)),
        trace=_trace,
        **(_trace_kwargs or {}),
    )
    out = np.concatenate(
        [res.results[c]["out"] for c in range(NCORES)], axis=1
    ).astype(np.float32)
    if _trace:
        kernel._last_result = res
    return out
